# revision 41
# baseline (speedup 1.0000x reference)
"""NCC loss (local normalized cross-correlation, window 9^3) on 8 Trainium2
NeuronCores.

Reference: 5 channels [I, J, I^2, J^2, IJ] box-filtered (separable 9-tap mean,
SAME zero-pad) over a 192^3 volume; cc = sigma12^2/(sigma1^2*sigma2^2+eps);
output = 1 - mean(cc).

Sharding: depth axis. Core c computes output slices [24c, 24c+24), reading
padded input slices [24c, 24c+32) of the (+4 both ends) padded volume.

Pipeline (numerics validated in model2.py; measured 245.8 us HW vs the
815 us transpose-based baseline):
  host   : shift x-0.5 (pad value -0.5 == shifted zero-pad; exact for NCC),
           cast bf16, interleave [targ|pred] rows -> one DMA lands I|J rows
           per partition.
  prep   : squares + product on DVE (TT 2x; keeps the ACT queue free for
           PSUM evacuations - ACT head-of-line blocking stalls the PSUM
           WAR chain otherwise).
  H pass : banded matmuls (TensorE, band=1/729) accumulated over slices into
           PSUM (cumsum over D); bf16 snapshots to SBUF (both on ACT -
           ScalarE is closer to PSUM and DVE is the bottleneck engine).
  D pass : window sum dd = C[oz+8]-C[oz-1] computed on TensorE as
           I*hi + (-I)*lo into PSUM banks 4-7, evacuated by ScalarE
           (frees ~28us of DVE; hw CCE has no subtract, and GPSIMD TT
           SBUF-port contention with 2-port DVE ops is a large net loss).
  W pass : 9-tap window sum along the free axis via ONE tensor_tensor_scan
           (s[t] = s[t-1] + dd[t] - dd[t-9], f32 state telescopes exactly)
           over a flat [9z|A-half|9z|B-half] layout - no transposes, no W
           matmuls.  (Scan measured 2 cyc/elem and is recurrence-bound: an
           op1=bypass single-operand scan is no faster.)
  cc     : elementwise DVE/ACT in one table set (square/copy only -> a
           single ACT_TABLE_LOAD); division via DVE reciprocal_approx_fast
           on (sg1+eps)*sg2 in f32; per-partition sums via stt accum_out,
           B-half rows 80..111 (structurally zero) excluded.
Host: 1 - sum(acc)/192^3.
"""

import sys

import numpy as np

sys.path.insert(0, "/opt/trn_rl_repo")

import contextlib

import concourse.bacc as bacc
import concourse.mybir as mybir
from concourse import tile
from concourse.bass_utils import run_bass_kernel_spmd

F32 = mybir.dt.float32
BF16 = mybir.dt.bfloat16
AOT = mybir.AluOpType
ACTF = mybir.ActivationFunctionType

H = 192
W = 192
D_TOT = 192
HE = 200   # extended h (4 pad each side)
WE = 200   # extended w
PAD = 4
N_CORES = 8

HA = 112   # H-pass out: ext rows 4..115  == orig h 0..111
HB = 80    # H-pass out: ext rows 116..195 == orig h 112..191
KT = 128   # chan A partitions: ext-h 0..127 (matmul uses 0..119)
KB = 88    # chan B partitions: ext-h 112..199

BAND_C = 1.0 / 729.0
NCH = 5
HALF = NCH * WE            # 1000 channel cols per h-chunk
CFREE = 2 * HALF           # 2000 (chan/snap tiles)
DDW = 2 * (9 + HALF)       # 2018 (diff tiles, 9 leading zeros per half)
SCW = DDW - 9              # 2009 scanned elements

EPS = float(np.finfo(np.float32).eps)


def _band(rows, cols, val):
    k = np.arange(rows)[:, None]
    m = np.arange(cols)[None, :]
    return np.where((k - m >= 0) & (k - m <= 8), val, 0.0).astype(np.float32)


def make_consts():
    import ml_dtypes

    band = _band(120, 112, BAND_C).astype(ml_dtypes.bfloat16)
    eye = np.eye(HA, dtype=np.float32)
    ident = np.concatenate([eye, -eye], axis=1).astype(ml_dtypes.bfloat16)
    return band, ident


def build_program(din, dout):
    nc = bacc.Bacc(
        "TRN2", target_bir_lowering=False, debug=False, num_devices=N_CORES
    )

    vol_d = nc.dram_tensor("vol", [din, HE, 2, WE], BF16, kind="ExternalInput")
    band_d = nc.dram_tensor("band", [120, 112], BF16, kind="ExternalInput")
    ident_d = nc.dram_tensor(
        "ident", [HA, 2 * HA], BF16, kind="ExternalInput"
    )
    out_d = nc.dram_tensor("out", [HA, 2 * dout], F32, kind="ExternalOutput")

    vol = vol_d.ap()

    with tile.TileContext(nc) as tc, contextlib.ExitStack() as ctx:
        consts = ctx.enter_context(tc.tile_pool(name="consts", bufs=1))
        chans = ctx.enter_context(tc.tile_pool(name="chans", bufs=4))
        snapsp = ctx.enter_context(tc.tile_pool(name="snaps", bufs=1))
        ddp = ctx.enter_context(tc.tile_pool(name="dds", bufs=1))
        wfs = ctx.enter_context(tc.tile_pool(name="wfs", bufs=3))
        ccs = ctx.enter_context(tc.tile_pool(name="ccs", bufs=3))
        accp = ctx.enter_context(tc.tile_pool(name="accp", bufs=1))
        ps_h = ctx.enter_context(tc.tile_pool(name="psh", bufs=1, space="PSUM"))

        band = consts.tile([120, 112], BF16, tag="band")
        nc.sync.dma_start(band[:], band_d.ap())
        ident = consts.tile([HA, 2 * HA], BF16, tag="ident")
        nc.sync.dma_start(ident[:], ident_d.ap())
        identP = ident[:, 0:HA]
        identN = ident[:, HA : 2 * HA]

        # H-cum PSUM; free padded to 1024 so each 500-piece sits in one bank
        psA = ps_h.tile([HA, 1024], F32, tag="psA")
        psB = ps_h.tile([HB, 1024], F32, tag="psB")
        psA3 = psA.rearrange("p (b w) -> p b w", b=2)  # [*, 2, 512]
        psB3 = psB.rearrange("p (b w) -> p b w", b=2)
        # D-diff PSUM (banks 4-7): dd = I*hi + (-I)*lo via TensorE.
        # Two tiles (one per h-chunk) so each evacuation only waits on its
        # own matmul group and the next oz's matmuls only on its evac.
        ddpsA = ps_h.tile([HA, 1024], F32, tag="ddpsA")
        ddpsB = ps_h.tile([HA, 1024], F32, tag="ddpsB")
        ddpsA3 = ddpsA.rearrange("p (b w) -> p b w", b=2)  # [112, 2, 512]
        ddpsB3 = ddpsB.rearrange("p (b w) -> p b w", b=2)

        acc = accp.tile([HA, 2 * dout], F32, tag="acc")
        nc.vector.memset(acc[:], 0.0)

        # persistent snapshot ring (zsnap = all-zero lo for oz==0); B half
        # rows 80..111 stay zero forever -> cc there is exactly 0.
        # one-time initialization memsets go to GPSIMD: it is idle, and at
        # startup there is no DVE traffic to contend with
        zsnap = consts.tile([HA, CFREE], BF16, tag="zsnap")
        nc.gpsimd.memset(zsnap[:], 0.0)
        NSNAP = 11
        snap_ring = []
        for i in range(NSNAP):
            s = snapsp.tile([HA, CFREE], BF16, tag=f"snap{i}", name=f"snap{i}")
            # only rows 80.. of the B half must be (and stay) zero; the rest
            # is fully rewritten every slice (partition start must be
            # 32-aligned, so start at 64)
            nc.gpsimd.memset(s[64:HA, HALF:CFREE], 0.0)
            snap_ring.append(s)

        # ping-pong diff tiles; cols 0..8 and 1009..1017 stay zero
        dd_tiles = []
        for i in range(3):
            t = ddp.tile([HA, DDW], BF16, tag=f"dd{i}", name=f"dd{i}")
            nc.gpsimd.memset(t[:, 0:9], 0.0)
            nc.gpsimd.memset(t[:, 9 + HALF : 18 + HALF], 0.0)
            dd_tiles.append(t)

        # chan tiles pre-zeroed so the first squares pass reads no
        # uninitialized SBUF in rows 88..127 of the B half
        chan_ring = []
        for i in range(4):
            t = chans.tile([KT, CFREE], BF16, tag=f"chan{i}", name=f"chan{i}")
            nc.gpsimd.memset(t[64:KT, HALF:CFREE], 0.0)
            chan_ring.append(t)

        def h_pass(z):
            chan = chan_ring[z % 4]
            # DMA lands [I row | J row] = ch0 | ch1 of each half
            nc.sync.dma_start(chan[0:KT, 0:400], vol[z, 0:KT])
            nc.sync.dma_start(chan[0:KB, HALF : HALF + 400], vol[z, HE - KB : HE])

            chh = chan.rearrange("p (h q) -> p h q", h=2)
            # ch2 = I^2, ch3 = J^2 on DVE (TT self-mult at 2x) to keep the
            # ACT queue free for PSUM evacuations
            nc.vector.tensor_tensor(
                chh[:, :, 400:800], chh[:, :, 0:400], chh[:, :, 0:400],
                AOT.mult,
            )
            # ch4 = I*J
            nc.vector.tensor_tensor(
                chh[:, :, 800:1000], chh[:, :, 0:200], chh[:, :, 200:400],
                AOT.mult,
            )

            start = z == 0
            nc.tensor.matmul(
                psA3[:, 0, 0:500], band[0:120, 0:HA], chan[0:120, 0:500],
                start=start, stop=True, skip_group_check=True,
            )
            nc.tensor.matmul(
                psA3[:, 1, 0:500], band[0:120, 0:HA], chan[0:120, 500:1000],
                start=start, stop=True, skip_group_check=True,
            )
            nc.tensor.matmul(
                psB3[:, 0, 0:500], band[0:KB, 0:HB],
                chan[0:KB, HALF : HALF + 500],
                start=start, stop=True, skip_group_check=True,
            )
            nc.tensor.matmul(
                psB3[:, 1, 0:500], band[0:KB, 0:HB],
                chan[0:KB, HALF + 500 : CFREE],
                start=start, stop=True, skip_group_check=True,
            )

            # both PSUM evacuations on ACT (closer to PSUM; DVE is the
            # bottleneck engine)
            snap = snap_ring[z % NSNAP]
            s4 = snap.rearrange("p (g b w) -> p g b w", g=2, b=2)
            nc.scalar.copy(s4[:, 0], psA3[:, :, 0:500])
            nc.scalar.copy(s4[0:HB, 1], psB3[:, :, 0:500])

        def w_pass(oz):
            hi = snap_ring[(oz + 8) % NSNAP]
            lo = zsnap if oz == 0 else snap_ring[(oz - 1) % NSNAP]
            dd = dd_tiles[oz % 3]

            # D window = C[oz+8] - C[oz-1], computed on the idle TensorE as
            # dd = I*hi + (-I)*lo accumulated in PSUM banks 4-7, evacuated
            # by ScalarE. Frees ~28us of DVE (the bottleneck engine).
            # hi/lo interleaved per bank so each bank's group closes as
            # early as possible and the evac can start after 4 matmuls.
            for t3, base in ((ddpsA3, 0), (ddpsB3, 2)):
                for p in range(2):
                    sl = slice((base + p) * 500, (base + p + 1) * 500)
                    nc.tensor.matmul(
                        t3[:, p, 0:500], identP, hi[:, sl],
                        start=True, stop=False, skip_group_check=True,
                    )
                    nc.tensor.matmul(
                        t3[:, p, 0:500], identN, lo[:, sl],
                        start=False, stop=True, skip_group_check=True,
                    )
            ddv1 = dd[:, 9 : 9 + HALF].rearrange("p (b w) -> p b w", b=2)
            ddv2 = dd[:, 18 + HALF : DDW].rearrange("p (b w) -> p b w", b=2)
            nc.scalar.copy(ddv1, ddpsA3[:, :, 0:500])
            nc.scalar.copy(ddv2, ddpsB3[:, :, 0:500])

            # W window sum: s[t] = s[t-1] + dd[t] - dd[t-9] (f32 state keeps
            # the telescoping exact; a pre-rounded bf16 g would random-walk)
            wf = wfs.tile([HA, DDW], BF16, tag="wf", name="wf")
            nc.vector.tensor_tensor_scan(
                wf[:, 0:SCW], dd[:, 9:DDW], dd[:, 0:SCW], 0.0,
                AOT.add, AOT.subtract,
            )

            wfh = wf.rearrange("p (h q) -> p h q", h=2)  # [112, 2, 1009]

            def F(c):
                return wfh[:, :, c * WE + 8 : c * WE + 200]

            sc = ccs.tile([HA, 4 * 384], BF16, tag="sc", name="sc")
            sc4 = sc.rearrange("p (s h x) -> p s h x", s=4, h=2)
            t1, s12, sg1, sg2 = (sc4[:, i] for i in range(4))
            sq = ccs.tile([HA, 2 * 384], BF16, tag="sq", name="sq")
            sq2v = sq.rearrange("p (s h x) -> p s h x", s=2, h=2)
            denf = ccs.tile([HA, 384], F32, tag="denf", name="denf")
            rdenf = ccs.tile([HA, 384], F32, tag="rdenf", name="rdenf")
            numf = ccs.tile([HA, 384], F32, tag="numf", name="numf")
            ccv = ccs.tile([HA, 384], BF16, tag="ccv", name="ccv")
            den3 = denf.rearrange("p (h x) -> p h x", h=2)
            rden3 = rdenf.rearrange("p (h x) -> p h x", h=2)
            num3 = numf.rearrange("p (h x) -> p h x", h=2)
            ccv3 = ccv.rearrange("p (h x) -> p h x", h=2)

            nc.vector.tensor_tensor(t1, F(0), F(1), AOT.mult)
            nc.vector.tensor_tensor(s12, F(4), t1, AOT.subtract)
            nc.scalar.activation(sq2v[:, 0], F(0), ACTF.Square)
            nc.scalar.activation(sq2v[:, 1], F(1), ACTF.Square)
            nc.vector.tensor_tensor(sg1, F(2), sq2v[:, 0], AOT.subtract)
            nc.vector.tensor_tensor(sg2, F(3), sq2v[:, 1], AOT.subtract)
            # den = (sg1 + eps) * sg2 > 0 on real rows; the all-zero B-half
            # rows 80..111 give den = 0 -> rden = NaN, but those rows are
            # excluded from the accumulation below.
            nc.vector.scalar_tensor_tensor(
                den3, sg1, EPS, sg2, AOT.add, AOT.mult
            )
            nc.vector.reciprocal_approx_fast(rdenf[:], denf[:])
            nc.scalar.activation(num3, s12, ACTF.Square)
            # cc = num * rden, accumulated per partition; halves separately
            # so the zero/garbage B-half rows 80..111 are never read
            nc.vector.scalar_tensor_tensor(
                ccv3[:, 0], num3[:, 0], 1.0, rden3[:, 0], AOT.mult, AOT.mult,
                accum_out=acc[:, 2 * oz : 2 * oz + 1],
            )
            nc.vector.scalar_tensor_tensor(
                ccv3[0:HB, 1], num3[0:HB, 1], 1.0, rden3[0:HB, 1],
                AOT.mult, AOT.mult,
                accum_out=acc[0:HB, 2 * oz + 1 : 2 * oz + 2],
            )

        for z in range(din):
            h_pass(z)
            oz = z - 8
            if 0 <= oz < dout:
                w_pass(oz)

        nc.sync.dma_start(out_d.ap(), acc[:])

    nc.compile()
    return nc


_PROGRAM_CACHE = {}


def _get_program(din, dout):
    key = (din, dout)
    if key not in _PROGRAM_CACHE:
        _PROGRAM_CACHE[key] = build_program(din, dout)
    return _PROGRAM_CACHE[key]


def kernel(pred, target):
    import ml_dtypes

    pred = np.asarray(pred).reshape(D_TOT, H, W).astype(np.float32)
    targ = np.asarray(target).reshape(D_TOT, H, W).astype(np.float32)

    dout = D_TOT // N_CORES
    din = dout + 2 * PAD

    # shifted (x - 0.5) volume; pad value -0.5 == shifted zero-pad (exact)
    shp = (D_TOT + 2 * PAD, HE, 2, WE)
    vol = np.full(shp, -0.5, np.float32)
    vol[PAD:-PAD, PAD : PAD + H, 0, PAD : PAD + W] = targ - 0.5  # I
    vol[PAD:-PAD, PAD : PAD + H, 1, PAD : PAD + W] = pred - 0.5  # J
    vol = vol.astype(ml_dtypes.bfloat16)

    band, ident = make_consts()
    nc = _get_program(din, dout)

    in_maps = []
    for c in range(N_CORES):
        s = c * dout
        in_maps.append(
            {
                "vol": np.ascontiguousarray(vol[s : s + din]),
                "band": band,
                "ident": ident,
            }
        )

    res = run_bass_kernel_spmd(nc, in_maps, core_ids=list(range(N_CORES)))
    total = sum(float(r["out"].astype(np.float64).sum()) for r in res.results)
    return np.float32(1.0 - total / float(D_TOT * H * W))


# revision 46
# speedup vs baseline: 1.0120x; 1.0120x over previous
"""NCC loss (local normalized cross-correlation, window 9^3) on 8 Trainium2
NeuronCores.

Reference: 5 channels [I, J, I^2, J^2, IJ] box-filtered (separable 9-tap mean,
SAME zero-pad) over a 192^3 volume; cc = sigma12^2/(sigma1^2*sigma2^2+eps);
output = 1 - mean(cc).

Sharding: depth axis. Core c computes output slices [24c, 24c+24), reading
padded input slices [24c, 24c+32) of the (+4 both ends) padded volume.

Pipeline (numerics validated in model2.py; measured 238.9 us HW vs the
815 us transpose-based baseline):
  host   : shift x-0.5 (pad value -0.5 == shifted zero-pad; exact for NCC),
           cast bf16, interleave [targ|pred] rows -> one DMA lands I|J rows
           per partition.
  prep   : squares + product on DVE (TT 2x; keeps the ACT queue free for
           PSUM evacuations - ACT head-of-line blocking stalls the PSUM
           WAR chain otherwise).
  H pass : banded matmuls (TensorE, band=1/729) accumulated over slices into
           PSUM (cumsum over D); bf16 snapshots to SBUF (both on ACT -
           ScalarE is closer to PSUM and DVE is the bottleneck engine).
  D pass : window sum dd = C[oz+8]-C[oz-1] computed on TensorE as
           I*hi + (-I)*lo into PSUM banks 4-7, evacuated by ScalarE
           (frees ~28us of DVE; hw CCE has no subtract, and GPSIMD TT
           SBUF-port contention with 2-port DVE ops is a large net loss).
  W pass : 9-tap window sum along the free axis via ONE tensor_tensor_scan
           (s[t] = s[t-1] + dd[t] - dd[t-9], f32 state telescopes exactly)
           over a flat [9z|A-half|9z|B-half] layout - no transposes, no W
           matmuls.  (Scan measured 2 cyc/elem and is recurrence-bound: an
           op1=bypass single-operand scan is no faster.)
  cc     : elementwise DVE/ACT in one table set (square/copy only -> a
           single ACT_TABLE_LOAD); division via DVE reciprocal_approx_fast
           on (sg1+eps)*sg2 in f32; per-partition sums via stt accum_out,
           B-half rows 80..111 (structurally zero) excluded.
Host: 1 - sum(acc)/192^3.
"""

import sys

import numpy as np

sys.path.insert(0, "/opt/trn_rl_repo")

import contextlib

import concourse.bacc as bacc
import concourse.mybir as mybir
from concourse import tile
from concourse.bass_utils import run_bass_kernel_spmd

F32 = mybir.dt.float32
BF16 = mybir.dt.bfloat16
AOT = mybir.AluOpType
ACTF = mybir.ActivationFunctionType

H = 192
W = 192
D_TOT = 192
HE = 200   # extended h (4 pad each side)
WE = 200   # extended w
PAD = 4
N_CORES = 8

HA = 112   # H-pass out: ext rows 4..115  == orig h 0..111
HB = 80    # H-pass out: ext rows 116..195 == orig h 112..191
KT = 128   # chan A partitions: ext-h 0..127 (matmul uses 0..119)
KB = 88    # chan B partitions: ext-h 112..199

BAND_C = 1.0 / 729.0
NCH = 5
HALF = NCH * WE            # 1000 channel cols per h-chunk
CFREE = 2 * HALF           # 2000 (chan/snap tiles)
DDW = 2 * (9 + HALF)       # 2018 (diff tiles, 9 leading zeros per half)
SCW = DDW - 9              # 2009 scanned elements

EPS = float(np.finfo(np.float32).eps)


def _band(rows, cols, val):
    k = np.arange(rows)[:, None]
    m = np.arange(cols)[None, :]
    return np.where((k - m >= 0) & (k - m <= 8), val, 0.0).astype(np.float32)


def make_consts():
    import ml_dtypes

    band = _band(120, 112, BAND_C).astype(ml_dtypes.bfloat16)
    eye = np.eye(HA, dtype=np.float32)
    ident = np.concatenate([eye, -eye], axis=1).astype(ml_dtypes.bfloat16)
    return band, ident


def build_program(din, dout):
    nc = bacc.Bacc(
        "TRN2", target_bir_lowering=False, debug=False, num_devices=N_CORES
    )

    vol_d = nc.dram_tensor(
        "vol", [din, HE, NCH, WE], BF16, kind="ExternalInput"
    )
    band_d = nc.dram_tensor("band", [120, 112], BF16, kind="ExternalInput")
    ident_d = nc.dram_tensor(
        "ident", [HA, 2 * HA], BF16, kind="ExternalInput"
    )
    out_d = nc.dram_tensor("out", [HA, 2 * dout], F32, kind="ExternalOutput")

    vol = vol_d.ap()

    with tile.TileContext(nc) as tc, contextlib.ExitStack() as ctx:
        consts = ctx.enter_context(tc.tile_pool(name="consts", bufs=1))
        chans = ctx.enter_context(tc.tile_pool(name="chans", bufs=4))
        snapsp = ctx.enter_context(tc.tile_pool(name="snaps", bufs=1))
        ddp = ctx.enter_context(tc.tile_pool(name="dds", bufs=1))
        wfs = ctx.enter_context(tc.tile_pool(name="wfs", bufs=3))
        ccs = ctx.enter_context(tc.tile_pool(name="ccs", bufs=3))
        accp = ctx.enter_context(tc.tile_pool(name="accp", bufs=1))
        ps_h = ctx.enter_context(tc.tile_pool(name="psh", bufs=1, space="PSUM"))

        band = consts.tile([120, 112], BF16, tag="band")
        nc.sync.dma_start(band[:], band_d.ap())
        ident = consts.tile([HA, 2 * HA], BF16, tag="ident")
        nc.sync.dma_start(ident[:], ident_d.ap())
        identP = ident[:, 0:HA]
        identN = ident[:, HA : 2 * HA]

        # H-cum PSUM; free padded to 1024 so each 500-piece sits in one bank
        psA = ps_h.tile([HA, 1024], F32, tag="psA")
        psB = ps_h.tile([HB, 1024], F32, tag="psB")
        psA3 = psA.rearrange("p (b w) -> p b w", b=2)  # [*, 2, 512]
        psB3 = psB.rearrange("p (b w) -> p b w", b=2)
        # D-diff PSUM (banks 4-7): dd = I*hi + (-I)*lo via TensorE.
        # Two tiles (one per h-chunk) so each evacuation only waits on its
        # own matmul group and the next oz's matmuls only on its evac.
        ddpsA = ps_h.tile([HA, 1024], F32, tag="ddpsA")
        ddpsB = ps_h.tile([HA, 1024], F32, tag="ddpsB")
        ddpsA3 = ddpsA.rearrange("p (b w) -> p b w", b=2)  # [112, 2, 512]
        ddpsB3 = ddpsB.rearrange("p (b w) -> p b w", b=2)

        acc = accp.tile([HA, 2 * dout], F32, tag="acc")
        nc.vector.memset(acc[:], 0.0)

        # persistent snapshot ring (zsnap = all-zero lo for oz==0); B half
        # rows 80..111 stay zero forever -> cc there is exactly 0.
        # one-time initialization memsets go to GPSIMD: it is idle, and at
        # startup there is no DVE traffic to contend with
        zsnap = consts.tile([HA, CFREE], BF16, tag="zsnap")
        nc.gpsimd.memset(zsnap[:], 0.0)
        NSNAP = 11
        snap_ring = []
        for i in range(NSNAP):
            s = snapsp.tile([HA, CFREE], BF16, tag=f"snap{i}", name=f"snap{i}")
            # only rows 80.. of the B half must be (and stay) zero; the rest
            # is fully rewritten every slice (partition start must be
            # 32-aligned, so start at 64)
            nc.gpsimd.memset(s[64:HA, HALF:CFREE], 0.0)
            snap_ring.append(s)

        # ping-pong diff tiles; cols 0..8 and 1009..1017 stay zero
        dd_tiles = []
        for i in range(2):
            t = ddp.tile([HA, DDW], BF16, tag=f"dd{i}", name=f"dd{i}")
            nc.gpsimd.memset(t[:, 0:9], 0.0)
            nc.gpsimd.memset(t[:, 9 + HALF : 18 + HALF], 0.0)
            dd_tiles.append(t)

        # chan tiles pre-zeroed so the first squares pass reads no
        # uninitialized SBUF in rows 88..127 of the B half
        chan_ring = []
        for i in range(4):
            t = chans.tile([KT, CFREE], BF16, tag=f"chan{i}", name=f"chan{i}")
            nc.gpsimd.memset(t[64:KT, HALF:CFREE], 0.0)
            chan_ring.append(t)

        def h_pass(z):
            chan = chan_ring[z % 4]
            # all 5 channels [I, J, I^2, J^2, IJ] are host-precomputed:
            # one DMA per h-chunk lands the full channel block (2.5x the
            # bytes of the 2-channel scheme, but the DMA engines are idle
            # and this deletes all per-slice DVE prep work)
            nc.sync.dma_start(chan[0:KT, 0:HALF], vol[z, 0:KT])
            nc.sync.dma_start(chan[0:KB, HALF:CFREE], vol[z, HE - KB : HE])

            start = z == 0
            nc.tensor.matmul(
                psA3[:, 0, 0:500], band[0:120, 0:HA], chan[0:120, 0:500],
                start=start, stop=True, skip_group_check=True,
            )
            nc.tensor.matmul(
                psA3[:, 1, 0:500], band[0:120, 0:HA], chan[0:120, 500:1000],
                start=start, stop=True, skip_group_check=True,
            )
            nc.tensor.matmul(
                psB3[:, 0, 0:500], band[0:KB, 0:HB],
                chan[0:KB, HALF : HALF + 500],
                start=start, stop=True, skip_group_check=True,
            )
            nc.tensor.matmul(
                psB3[:, 1, 0:500], band[0:KB, 0:HB],
                chan[0:KB, HALF + 500 : CFREE],
                start=start, stop=True, skip_group_check=True,
            )

            # both PSUM evacuations on ACT (closer to PSUM; DVE is the
            # bottleneck engine)
            snap = snap_ring[z % NSNAP]
            s4 = snap.rearrange("p (g b w) -> p g b w", g=2, b=2)
            nc.scalar.copy(s4[:, 0], psA3[:, :, 0:500])
            nc.scalar.copy(s4[0:HB, 1], psB3[:, :, 0:500])

        def w_pass(oz):
            hi = snap_ring[(oz + 8) % NSNAP]
            lo = zsnap if oz == 0 else snap_ring[(oz - 1) % NSNAP]
            dd = dd_tiles[oz % 2]

            # D window = C[oz+8] - C[oz-1], computed on the idle TensorE as
            # dd = I*hi + (-I)*lo accumulated in PSUM banks 4-7, evacuated
            # by ScalarE. Frees ~28us of DVE (the bottleneck engine).
            # hi/lo interleaved per bank so each bank's group closes as
            # early as possible and the evac can start after 4 matmuls.
            for t3, base in ((ddpsA3, 0), (ddpsB3, 2)):
                for p in range(2):
                    sl = slice((base + p) * 500, (base + p + 1) * 500)
                    nc.tensor.matmul(
                        t3[:, p, 0:500], identP, hi[:, sl],
                        start=True, stop=False, skip_group_check=True,
                    )
                    nc.tensor.matmul(
                        t3[:, p, 0:500], identN, lo[:, sl],
                        start=False, stop=True, skip_group_check=True,
                    )
            ddv1 = dd[:, 9 : 9 + HALF].rearrange("p (b w) -> p b w", b=2)
            ddv2 = dd[:, 18 + HALF : DDW].rearrange("p (b w) -> p b w", b=2)
            nc.scalar.copy(ddv1, ddpsA3[:, :, 0:500])
            nc.scalar.copy(ddv2, ddpsB3[:, :, 0:500])

            # W window sum: s[t] = s[t-1] + dd[t] - dd[t-9] (f32 state keeps
            # the telescoping exact; a pre-rounded bf16 g would random-walk)
            wf = wfs.tile([HA, DDW], BF16, tag="wf", name="wf")
            nc.vector.tensor_tensor_scan(
                wf[:, 0:SCW], dd[:, 9:DDW], dd[:, 0:SCW], 0.0,
                AOT.add, AOT.subtract,
            )

            wfh = wf.rearrange("p (h q) -> p h q", h=2)  # [112, 2, 1009]

            def F(c):
                return wfh[:, :, c * WE + 8 : c * WE + 200]

            sc = ccs.tile([HA, 4 * 384], BF16, tag="sc", name="sc")
            sc4 = sc.rearrange("p (s h x) -> p s h x", s=4, h=2)
            t1, s12, sg1, sg2 = (sc4[:, i] for i in range(4))
            sq = ccs.tile([HA, 2 * 384], BF16, tag="sq", name="sq")
            sq2v = sq.rearrange("p (s h x) -> p s h x", s=2, h=2)
            denf = ccs.tile([HA, 384], F32, tag="denf", name="denf")
            rdenf = ccs.tile([HA, 384], F32, tag="rdenf", name="rdenf")
            numf = ccs.tile([HA, 384], F32, tag="numf", name="numf")
            ccv = ccs.tile([HA, 384], BF16, tag="ccv", name="ccv")
            den3 = denf.rearrange("p (h x) -> p h x", h=2)
            rden3 = rdenf.rearrange("p (h x) -> p h x", h=2)
            num3 = numf.rearrange("p (h x) -> p h x", h=2)
            ccv3 = ccv.rearrange("p (h x) -> p h x", h=2)

            nc.vector.tensor_tensor(t1, F(0), F(1), AOT.mult)
            nc.vector.tensor_tensor(s12, F(4), t1, AOT.subtract)
            nc.scalar.activation(sq2v[:, 0], F(0), ACTF.Square)
            nc.scalar.activation(sq2v[:, 1], F(1), ACTF.Square)
            nc.vector.tensor_tensor(sg1, F(2), sq2v[:, 0], AOT.subtract)
            nc.vector.tensor_tensor(sg2, F(3), sq2v[:, 1], AOT.subtract)
            # den = (sg1 + eps) * sg2 > 0 on real rows; the all-zero B-half
            # rows 80..111 give den = 0 -> rden = NaN, but those rows are
            # excluded from the accumulation below.
            nc.vector.scalar_tensor_tensor(
                den3, sg1, EPS, sg2, AOT.add, AOT.mult
            )
            nc.vector.reciprocal_approx_fast(rdenf[:], denf[:])
            nc.scalar.activation(num3, s12, ACTF.Square)
            # cc = num * rden, accumulated per partition; halves separately
            # so the zero/garbage B-half rows 80..111 are never read
            nc.vector.scalar_tensor_tensor(
                ccv3[:, 0], num3[:, 0], 1.0, rden3[:, 0], AOT.mult, AOT.mult,
                accum_out=acc[:, 2 * oz : 2 * oz + 1],
            )
            nc.vector.scalar_tensor_tensor(
                ccv3[0:HB, 1], num3[0:HB, 1], 1.0, rden3[0:HB, 1],
                AOT.mult, AOT.mult,
                accum_out=acc[0:HB, 2 * oz + 1 : 2 * oz + 2],
            )

        for z in range(din):
            h_pass(z)
            oz = z - 8
            if 0 <= oz < dout:
                w_pass(oz)

        nc.sync.dma_start(out_d.ap(), acc[:])

    nc.compile()
    return nc


_PROGRAM_CACHE = {}


def _get_program(din, dout):
    key = (din, dout)
    if key not in _PROGRAM_CACHE:
        _PROGRAM_CACHE[key] = build_program(din, dout)
    return _PROGRAM_CACHE[key]


def kernel(pred, target):
    import ml_dtypes

    pred = np.asarray(pred).reshape(D_TOT, H, W).astype(np.float32)
    targ = np.asarray(target).reshape(D_TOT, H, W).astype(np.float32)

    dout = D_TOT // N_CORES
    din = dout + 2 * PAD

    # shifted (x - 0.5) volume; pad value -0.5 == shifted zero-pad (exact).
    # All 5 channels are precomputed here (squaring the bf16-rounded values
    # bit-matches the previous on-chip prep).
    dpad = D_TOT + 2 * PAD
    Ip = np.full((dpad, HE, WE), -0.5, np.float32)
    Jp = np.full((dpad, HE, WE), -0.5, np.float32)
    Ip[PAD:-PAD, PAD : PAD + H, PAD : PAD + W] = targ - 0.5
    Jp[PAD:-PAD, PAD : PAD + H, PAD : PAD + W] = pred - 0.5
    Ib = Ip.astype(ml_dtypes.bfloat16)
    Jb = Jp.astype(ml_dtypes.bfloat16)
    If = Ib.astype(np.float32)
    Jf = Jb.astype(np.float32)
    vol = np.empty((dpad, HE, NCH, WE), ml_dtypes.bfloat16)
    vol[:, :, 0] = Ib
    vol[:, :, 1] = Jb
    vol[:, :, 2] = (If * If).astype(ml_dtypes.bfloat16)
    vol[:, :, 3] = (Jf * Jf).astype(ml_dtypes.bfloat16)
    vol[:, :, 4] = (If * Jf).astype(ml_dtypes.bfloat16)

    band, ident = make_consts()
    nc = _get_program(din, dout)

    in_maps = []
    for c in range(N_CORES):
        s = c * dout
        in_maps.append(
            {
                "vol": np.ascontiguousarray(vol[s : s + din]),
                "band": band,
                "ident": ident,
            }
        )

    res = run_bass_kernel_spmd(nc, in_maps, core_ids=list(range(N_CORES)))
    total = sum(float(r["out"].astype(np.float64).sum()) for r in res.results)
    return np.float32(1.0 - total / float(D_TOT * H * W))


# revision 50
# speedup vs baseline: 1.0684x; 1.0557x over previous
"""NCC loss (local normalized cross-correlation, window 9^3) on 8 Trainium2
NeuronCores.

Reference: 5 channels [I, J, I^2, J^2, IJ] box-filtered (separable 9-tap mean,
SAME zero-pad) over a 192^3 volume; cc = sigma12^2/(sigma1^2*sigma2^2+eps);
output = 1 - mean(cc).

Sharding: depth axis. Core c computes output slices [24c, 24c+24), reading
padded input slices [24c, 24c+32) of the (+4 both ends) padded volume.

Pipeline (numerics validated in model2.py; measured 237.3 us HW vs the
815 us transpose-based baseline):
  host   : shift x-0.5 (pad value -0.5 == shifted zero-pad; exact for NCC),
           precompute ALL FIVE channels [I, J, I^2, J^2, IJ] in bf16 -
           2.5x the DMA bytes (engines idle) but zero on-chip prep;
           one DMA per h-chunk lands the full channel block.
  H pass : banded matmuls (TensorE, band=1/729) accumulated over slices into
           PSUM (cumsum over D); bf16 snapshots to SBUF (both on ACT -
           ScalarE is closer to PSUM and DVE is the bottleneck engine).
  D pass : window sum dd = C[oz+8]-C[oz-1] computed on TensorE as
           I*hi + (-I)*lo into PSUM banks 4-7, evacuated by ScalarE
           (frees ~28us of DVE; hw CCE has no subtract, and GPSIMD TT
           SBUF-port contention with 2-port DVE ops is a large net loss).
  W pass : 9-tap window sum along the free axis via ONE tensor_tensor_scan
           (s[t] = s[t-1] + dd[t] - dd[t-9], f32 state telescopes exactly)
           over a flat [9z|A-half|9z|B-half] layout - no transposes, no W
           matmuls.  (Scan measured 2 cyc/elem and is recurrence-bound: an
           op1=bypass single-operand scan is no faster.)
  cc     : elementwise DVE/ACT in one table set (square/copy only -> a
           single ACT_TABLE_LOAD); division via DVE reciprocal_approx_fast
           on (sg1+eps)*sg2 in f32; per-partition sums via stt accum_out,
           B-half rows 80..111 (structurally zero) excluded.
Host: 1 - sum(acc)/192^3.
"""

import sys

import numpy as np

sys.path.insert(0, "/opt/trn_rl_repo")

import contextlib

import concourse.bacc as bacc
import concourse.mybir as mybir
from concourse import tile
from concourse.bass_utils import run_bass_kernel_spmd

F32 = mybir.dt.float32
BF16 = mybir.dt.bfloat16
AOT = mybir.AluOpType
ACTF = mybir.ActivationFunctionType

H = 192
W = 192
D_TOT = 192
HE = 200   # extended h (4 pad each side)
WE = 200   # extended w
PAD = 4
N_CORES = 8

HA = 112   # H-pass out: ext rows 4..115  == orig h 0..111
HB = 80    # H-pass out: ext rows 116..195 == orig h 112..191
KT = 128   # chan A partitions: ext-h 0..127 (matmul uses 0..119)
KB = 88    # chan B partitions: ext-h 112..199

BAND_C = 1.0 / 729.0
NCH = 5
HALF = NCH * WE            # 1000 channel cols per h-chunk
CFREE = 2 * HALF           # 2000 (chan/snap tiles)
DDW = 2 * (9 + HALF)       # 2018 (diff tiles, 9 leading zeros per half)
SCW = DDW - 9              # 2009 scanned elements

EPS = float(np.finfo(np.float32).eps)


def _band(rows, cols, val):
    k = np.arange(rows)[:, None]
    m = np.arange(cols)[None, :]
    return np.where((k - m >= 0) & (k - m <= 8), val, 0.0).astype(np.float32)


def make_consts():
    import ml_dtypes

    band = _band(120, 112, BAND_C).astype(ml_dtypes.bfloat16)
    eye = np.eye(HA, dtype=np.float32)
    ident = np.concatenate([eye, -eye], axis=1).astype(ml_dtypes.bfloat16)
    return band, ident


def build_program(din, dout):
    nc = bacc.Bacc(
        "TRN2", target_bir_lowering=False, debug=False, num_devices=N_CORES
    )

    vol_d = nc.dram_tensor(
        "vol", [din, HE, NCH, WE], BF16, kind="ExternalInput"
    )
    band_d = nc.dram_tensor("band", [120, 112], BF16, kind="ExternalInput")
    ident_d = nc.dram_tensor(
        "ident", [HA, 2 * HA], BF16, kind="ExternalInput"
    )
    out_d = nc.dram_tensor("out", [HA, 2 * dout], F32, kind="ExternalOutput")

    vol = vol_d.ap()

    with tile.TileContext(nc) as tc, contextlib.ExitStack() as ctx:
        consts = ctx.enter_context(tc.tile_pool(name="consts", bufs=1))
        chans = ctx.enter_context(tc.tile_pool(name="chans", bufs=4))
        snapsp = ctx.enter_context(tc.tile_pool(name="snaps", bufs=1))
        ddp = ctx.enter_context(tc.tile_pool(name="dds", bufs=1))
        wfs = ctx.enter_context(tc.tile_pool(name="wfs", bufs=3))
        ccs = ctx.enter_context(tc.tile_pool(name="ccs", bufs=3))
        accp = ctx.enter_context(tc.tile_pool(name="accp", bufs=1))
        ps_h = ctx.enter_context(tc.tile_pool(name="psh", bufs=1, space="PSUM"))

        band = consts.tile([120, 112], BF16, tag="band")
        nc.sync.dma_start(band[:], band_d.ap())
        ident = consts.tile([HA, 2 * HA], BF16, tag="ident")
        nc.sync.dma_start(ident[:], ident_d.ap())
        identP = ident[:, 0:HA]
        identN = ident[:, HA : 2 * HA]

        # H-cum PSUM; free padded to 1024 so each 500-piece sits in one bank
        psA = ps_h.tile([HA, 1024], F32, tag="psA")
        psB = ps_h.tile([HB, 1024], F32, tag="psB")
        psA3 = psA.rearrange("p (b w) -> p b w", b=2)  # [*, 2, 512]
        psB3 = psB.rearrange("p (b w) -> p b w", b=2)
        # D-diff PSUM (banks 4-7): dd = I*hi + (-I)*lo via TensorE.
        # Two tiles (one per h-chunk) so each evacuation only waits on its
        # own matmul group and the next oz's matmuls only on its evac.
        ddpsA = ps_h.tile([HA, 1024], F32, tag="ddpsA")
        ddpsB = ps_h.tile([HA, 1024], F32, tag="ddpsB")
        ddpsA3 = ddpsA.rearrange("p (b w) -> p b w", b=2)  # [112, 2, 512]
        ddpsB3 = ddpsB.rearrange("p (b w) -> p b w", b=2)

        acc = accp.tile([HA, 2 * dout], F32, tag="acc")
        nc.vector.memset(acc[:], 0.0)

        # persistent snapshot ring (zsnap = all-zero lo for oz==0); B half
        # rows 80..111 stay zero forever -> cc there is exactly 0.
        # one-time initialization memsets go to GPSIMD: it is idle, and at
        # startup there is no DVE traffic to contend with
        zsnap = consts.tile([HA, CFREE], BF16, tag="zsnap")
        nc.gpsimd.memset(zsnap[:], 0.0)
        NSNAP = 11
        snap_ring = []
        for i in range(NSNAP):
            s = snapsp.tile([HA, CFREE], BF16, tag=f"snap{i}", name=f"snap{i}")
            # only rows 80.. of the B half must be (and stay) zero; the rest
            # is fully rewritten every slice (partition start must be
            # 32-aligned, so start at 64)
            nc.gpsimd.memset(s[64:HA, HALF:CFREE], 0.0)
            snap_ring.append(s)

        # ping-pong diff tiles; cols 0..8 and 1009..1017 stay zero
        dd_tiles = []
        for i in range(2):
            t = ddp.tile([HA, DDW], BF16, tag=f"dd{i}", name=f"dd{i}")
            nc.gpsimd.memset(t[:, 0:9], 0.0)
            nc.gpsimd.memset(t[:, 9 + HALF : 18 + HALF], 0.0)
            dd_tiles.append(t)

        # chan tiles pre-zeroed so the first squares pass reads no
        # uninitialized SBUF in rows 88..127 of the B half
        chan_ring = []
        for i in range(4):
            t = chans.tile([KT, CFREE], BF16, tag=f"chan{i}", name=f"chan{i}")
            nc.gpsimd.memset(t[64:KT, HALF:CFREE], 0.0)
            chan_ring.append(t)

        def h_pass(z):
            chan = chan_ring[z % 4]
            # all 5 channels [I, J, I^2, J^2, IJ] are host-precomputed:
            # one DMA per h-chunk lands the full channel block (2.5x the
            # bytes of the 2-channel scheme, but the DMA engines are idle
            # and this deletes all per-slice DVE prep work)
            nc.sync.dma_start(chan[0:KT, 0:HALF], vol[z, 0:KT])
            nc.sync.dma_start(chan[0:KB, HALF:CFREE], vol[z, HE - KB : HE])

            start = z == 0
            nc.tensor.matmul(
                psA3[:, 0, 0:500], band[0:120, 0:HA], chan[0:120, 0:500],
                start=start, stop=True, skip_group_check=True,
            )
            nc.tensor.matmul(
                psA3[:, 1, 0:500], band[0:120, 0:HA], chan[0:120, 500:1000],
                start=start, stop=True, skip_group_check=True,
            )
            nc.tensor.matmul(
                psB3[:, 0, 0:500], band[0:KB, 0:HB],
                chan[0:KB, HALF : HALF + 500],
                start=start, stop=True, skip_group_check=True,
            )
            nc.tensor.matmul(
                psB3[:, 1, 0:500], band[0:KB, 0:HB],
                chan[0:KB, HALF + 500 : CFREE],
                start=start, stop=True, skip_group_check=True,
            )

            # both PSUM evacuations on ACT (closer to PSUM; DVE is the
            # bottleneck engine)
            snap = snap_ring[z % NSNAP]
            s4 = snap.rearrange("p (g b w) -> p g b w", g=2, b=2)
            nc.scalar.copy(s4[:, 0], psA3[:, :, 0:500])
            nc.scalar.copy(s4[0:HB, 1], psB3[:, :, 0:500])

        wf_ring = {}

        def w_pass(oz):
            hi = snap_ring[(oz + 8) % NSNAP]
            lo = zsnap if oz == 0 else snap_ring[(oz - 1) % NSNAP]
            dd = dd_tiles[oz % 2]

            # D window = C[oz+8] - C[oz-1], computed on the idle TensorE as
            # dd = I*hi + (-I)*lo accumulated in PSUM banks 4-7, evacuated
            # by ScalarE. Frees ~28us of DVE (the bottleneck engine).
            # hi/lo interleaved per bank so each bank's group closes as
            # early as possible and the evac can start after 4 matmuls.
            for t3, base in ((ddpsA3, 0), (ddpsB3, 2)):
                for p in range(2):
                    sl = slice((base + p) * 500, (base + p + 1) * 500)
                    nc.tensor.matmul(
                        t3[:, p, 0:500], identP, hi[:, sl],
                        start=True, stop=False, skip_group_check=True,
                    )
                    nc.tensor.matmul(
                        t3[:, p, 0:500], identN, lo[:, sl],
                        start=False, stop=True, skip_group_check=True,
                    )
            ddv1 = dd[:, 9 : 9 + HALF].rearrange("p (b w) -> p b w", b=2)
            ddv2 = dd[:, 18 + HALF : DDW].rearrange("p (b w) -> p b w", b=2)
            nc.scalar.copy(ddv1, ddpsA3[:, :, 0:500])
            nc.scalar.copy(ddv2, ddpsB3[:, :, 0:500])

            # W window sum: s[t] = s[t-1] + dd[t] - dd[t-9] (f32 state keeps
            # the telescoping exact; a pre-rounded bf16 g would random-walk)
            wf = wfs.tile([HA, DDW], BF16, tag="wf", name="wf")
            nc.vector.tensor_tensor_scan(
                wf[:, 0:SCW], dd[:, 9:DDW], dd[:, 0:SCW], 0.0,
                AOT.add, AOT.subtract,
            )

            wf_ring[oz % 3] = wf

        def cc_pass(oz):
            # software-pipelined one iteration behind the scan, so these
            # ACT ops never head-of-line-block the next slice's snapshot
            # copies while waiting on an in-flight scan
            wf = wf_ring[oz % 3]
            wfh = wf.rearrange("p (h q) -> p h q", h=2)  # [112, 2, 1009]

            def F(c):
                return wfh[:, :, c * WE + 8 : c * WE + 200]

            sc = ccs.tile([HA, 4 * 384], BF16, tag="sc", name="sc")
            sc4 = sc.rearrange("p (s h x) -> p s h x", s=4, h=2)
            t1, s12, sg1, sg2 = (sc4[:, i] for i in range(4))
            sq = ccs.tile([HA, 2 * 384], BF16, tag="sq", name="sq")
            sq2v = sq.rearrange("p (s h x) -> p s h x", s=2, h=2)
            denf = ccs.tile([HA, 384], F32, tag="denf", name="denf")
            rdenf = ccs.tile([HA, 384], F32, tag="rdenf", name="rdenf")
            numf = ccs.tile([HA, 384], F32, tag="numf", name="numf")
            ccv = ccs.tile([HA, 384], BF16, tag="ccv", name="ccv")
            den3 = denf.rearrange("p (h x) -> p h x", h=2)
            rden3 = rdenf.rearrange("p (h x) -> p h x", h=2)
            num3 = numf.rearrange("p (h x) -> p h x", h=2)
            ccv3 = ccv.rearrange("p (h x) -> p h x", h=2)

            nc.vector.tensor_tensor(t1, F(0), F(1), AOT.mult)
            nc.vector.tensor_tensor(s12, F(4), t1, AOT.subtract)
            nc.scalar.activation(sq2v[:, 0], F(0), ACTF.Square)
            nc.scalar.activation(sq2v[:, 1], F(1), ACTF.Square)
            nc.vector.tensor_tensor(sg1, F(2), sq2v[:, 0], AOT.subtract)
            nc.vector.tensor_tensor(sg2, F(3), sq2v[:, 1], AOT.subtract)
            # den = (sg1 + eps) * sg2 > 0 on real rows; the all-zero B-half
            # rows 80..111 give den = 0 -> rden = NaN, but those rows are
            # excluded from the accumulation below.
            nc.vector.scalar_tensor_tensor(
                den3, sg1, EPS, sg2, AOT.add, AOT.mult
            )
            nc.vector.reciprocal_approx_fast(rdenf[:], denf[:])
            nc.scalar.activation(num3, s12, ACTF.Square)
            # cc = num * rden, accumulated per partition; halves separately
            # so the zero/garbage B-half rows 80..111 are never read
            nc.vector.scalar_tensor_tensor(
                ccv3[:, 0], num3[:, 0], 1.0, rden3[:, 0], AOT.mult, AOT.mult,
                accum_out=acc[:, 2 * oz : 2 * oz + 1],
            )
            nc.vector.scalar_tensor_tensor(
                ccv3[0:HB, 1], num3[0:HB, 1], 1.0, rden3[0:HB, 1],
                AOT.mult, AOT.mult,
                accum_out=acc[0:HB, 2 * oz + 1 : 2 * oz + 2],
            )

        for z in range(din):
            h_pass(z)
            oz = z - 8
            if 0 <= oz < dout:
                w_pass(oz)
            if 1 <= oz:
                cc_pass(oz - 1)
        cc_pass(dout - 1)

        nc.sync.dma_start(out_d.ap(), acc[:])

    nc.compile()
    return nc


_PROGRAM_CACHE = {}


def _get_program(din, dout):
    key = (din, dout)
    if key not in _PROGRAM_CACHE:
        _PROGRAM_CACHE[key] = build_program(din, dout)
    return _PROGRAM_CACHE[key]


def kernel(pred, target):
    import ml_dtypes

    pred = np.asarray(pred).reshape(D_TOT, H, W).astype(np.float32)
    targ = np.asarray(target).reshape(D_TOT, H, W).astype(np.float32)

    dout = D_TOT // N_CORES
    din = dout + 2 * PAD

    # shifted (x - 0.5) volume; pad value -0.5 == shifted zero-pad (exact).
    # All 5 channels are precomputed here (squaring the bf16-rounded values
    # bit-matches the previous on-chip prep).
    dpad = D_TOT + 2 * PAD
    Ip = np.full((dpad, HE, WE), -0.5, np.float32)
    Jp = np.full((dpad, HE, WE), -0.5, np.float32)
    Ip[PAD:-PAD, PAD : PAD + H, PAD : PAD + W] = targ - 0.5
    Jp[PAD:-PAD, PAD : PAD + H, PAD : PAD + W] = pred - 0.5
    Ib = Ip.astype(ml_dtypes.bfloat16)
    Jb = Jp.astype(ml_dtypes.bfloat16)
    If = Ib.astype(np.float32)
    Jf = Jb.astype(np.float32)
    vol = np.empty((dpad, HE, NCH, WE), ml_dtypes.bfloat16)
    vol[:, :, 0] = Ib
    vol[:, :, 1] = Jb
    vol[:, :, 2] = (If * If).astype(ml_dtypes.bfloat16)
    vol[:, :, 3] = (Jf * Jf).astype(ml_dtypes.bfloat16)
    vol[:, :, 4] = (If * Jf).astype(ml_dtypes.bfloat16)

    band, ident = make_consts()
    nc = _get_program(din, dout)

    in_maps = []
    for c in range(N_CORES):
        s = c * dout
        in_maps.append(
            {
                "vol": np.ascontiguousarray(vol[s : s + din]),
                "band": band,
                "ident": ident,
            }
        )

    res = run_bass_kernel_spmd(nc, in_maps, core_ids=list(range(N_CORES)))
    total = sum(float(r["out"].astype(np.float64).sum()) for r in res.results)
    return np.float32(1.0 - total / float(D_TOT * H * W))


# revision 52
# speedup vs baseline: 1.0871x; 1.0176x over previous
"""NCC loss (local normalized cross-correlation, window 9^3) on 8 Trainium2
NeuronCores.

Reference: 5 channels [I, J, I^2, J^2, IJ] box-filtered (separable 9-tap mean,
SAME zero-pad) over a 192^3 volume; cc = sigma12^2/(sigma1^2*sigma2^2+eps);
output = 1 - mean(cc).

Sharding: depth axis. Core c computes output slices [24c, 24c+24), reading
padded input slices [24c, 24c+32) of the (+4 both ends) padded volume.

Pipeline (numerics validated in model2.py; measured 237.3 us HW vs the
815 us transpose-based baseline):
  host   : shift x-0.5 (pad value -0.5 == shifted zero-pad; exact for NCC),
           precompute ALL FIVE channels [I, J, I^2, J^2, IJ] in bf16 -
           2.5x the DMA bytes (engines idle) but zero on-chip prep;
           one DMA per h-chunk lands the full channel block.
  H pass : banded matmuls (TensorE, band=1/729) accumulated over slices into
           PSUM (cumsum over D); bf16 snapshots to SBUF (both on ACT -
           ScalarE is closer to PSUM and DVE is the bottleneck engine).
  D pass : window sum dd = C[oz+8]-C[oz-1] computed on TensorE as
           I*hi + (-I)*lo into PSUM banks 4-7, evacuated by ScalarE
           (frees ~28us of DVE; hw CCE has no subtract, and GPSIMD TT
           SBUF-port contention with 2-port DVE ops is a large net loss).
  W pass : 9-tap window sum along the free axis via ONE tensor_tensor_scan
           (s[t] = s[t-1] + dd[t] - dd[t-9], f32 state telescopes exactly)
           over a flat [9z|A-half|9z|B-half] layout - no transposes, no W
           matmuls.  (Scan measured 2 cyc/elem and is recurrence-bound: an
           op1=bypass single-operand scan is no faster.)
  cc     : elementwise DVE/ACT in one table set (square/copy only -> a
           single ACT_TABLE_LOAD); division via DVE reciprocal_approx_fast
           on (sg1+eps)*sg2 in f32; per-partition sums via stt accum_out,
           B-half rows 80..111 (structurally zero) excluded.
Host: 1 - sum(acc)/192^3.
"""

import sys

import numpy as np

sys.path.insert(0, "/opt/trn_rl_repo")

import contextlib

import concourse.bacc as bacc
import concourse.mybir as mybir
from concourse import tile
from concourse.bass_utils import run_bass_kernel_spmd

F32 = mybir.dt.float32
BF16 = mybir.dt.bfloat16
AOT = mybir.AluOpType
ACTF = mybir.ActivationFunctionType

H = 192
W = 192
D_TOT = 192
HE = 200   # extended h (4 pad each side)
WE = 200   # extended w
PAD = 4
N_CORES = 8

HA = 112   # H-pass out: ext rows 4..115  == orig h 0..111
HB = 80    # H-pass out: ext rows 116..195 == orig h 112..191
KT = 128   # chan A partitions: ext-h 0..127 (matmul uses 0..119)
KB = 88    # chan B partitions: ext-h 112..199

BAND_C = 1.0 / 729.0
NCH = 5
HALF = NCH * WE            # 1000 channel cols per h-chunk
CFREE = 2 * HALF           # 2000 (chan/snap tiles)
DDW = 2 * (9 + HALF)       # 2018 (diff tiles, 9 leading zeros per half)
SCW = DDW - 9              # 2009 scanned elements

EPS = float(np.finfo(np.float32).eps)


def _band(rows, cols, val):
    k = np.arange(rows)[:, None]
    m = np.arange(cols)[None, :]
    return np.where((k - m >= 0) & (k - m <= 8), val, 0.0).astype(np.float32)


def make_consts():
    import ml_dtypes

    band = _band(120, 112, BAND_C).astype(ml_dtypes.bfloat16)
    eye = np.eye(HA, dtype=np.float32)
    ident = np.concatenate([eye, -eye], axis=1).astype(ml_dtypes.bfloat16)
    return band, ident


def build_program(din, dout):
    nc = bacc.Bacc(
        "TRN2", target_bir_lowering=False, debug=False, num_devices=N_CORES
    )

    vol_d = nc.dram_tensor(
        "vol", [din, HE, NCH, WE], BF16, kind="ExternalInput"
    )
    band_d = nc.dram_tensor("band", [120, 112], BF16, kind="ExternalInput")
    ident_d = nc.dram_tensor(
        "ident", [HA, 2 * HA], BF16, kind="ExternalInput"
    )
    out_d = nc.dram_tensor("out", [HA, 2 * dout], F32, kind="ExternalOutput")

    vol = vol_d.ap()

    with tile.TileContext(nc) as tc, contextlib.ExitStack() as ctx:
        consts = ctx.enter_context(tc.tile_pool(name="consts", bufs=1))
        chans = ctx.enter_context(tc.tile_pool(name="chans", bufs=4))
        snapsp = ctx.enter_context(tc.tile_pool(name="snaps", bufs=1))
        ddp = ctx.enter_context(tc.tile_pool(name="dds", bufs=1))
        wfs = ctx.enter_context(tc.tile_pool(name="wfs", bufs=3))
        ccs = ctx.enter_context(tc.tile_pool(name="ccs", bufs=3))
        accp = ctx.enter_context(tc.tile_pool(name="accp", bufs=1))
        ps_h = ctx.enter_context(tc.tile_pool(name="psh", bufs=1, space="PSUM"))

        band = consts.tile([120, 112], BF16, tag="band")
        nc.sync.dma_start(band[:], band_d.ap())
        ident = consts.tile([HA, 2 * HA], BF16, tag="ident")
        nc.sync.dma_start(ident[:], ident_d.ap())
        identP = ident[:, 0:HA]
        identN = ident[:, HA : 2 * HA]

        # H-cum PSUM; free padded to 1024 so each 500-piece sits in one bank
        psA = ps_h.tile([HA, 1024], F32, tag="psA")
        psB = ps_h.tile([HB, 1024], F32, tag="psB")
        psA3 = psA.rearrange("p (b w) -> p b w", b=2)  # [*, 2, 512]
        psB3 = psB.rearrange("p (b w) -> p b w", b=2)
        # D-diff PSUM (banks 4-7): dd = I*hi + (-I)*lo via TensorE.
        # Two tiles (one per h-chunk) so each evacuation only waits on its
        # own matmul group and the next oz's matmuls only on its evac.
        ddpsA = ps_h.tile([HA, 1024], F32, tag="ddpsA")
        ddpsB = ps_h.tile([HA, 1024], F32, tag="ddpsB")
        ddpsA3 = ddpsA.rearrange("p (b w) -> p b w", b=2)  # [112, 2, 512]
        ddpsB3 = ddpsB.rearrange("p (b w) -> p b w", b=2)

        acc = accp.tile([HA, 2 * dout], F32, tag="acc")
        nc.vector.memset(acc[:], 0.0)

        # persistent snapshot ring (zsnap = all-zero lo for oz==0); B half
        # rows 80..111 stay zero forever -> cc there is exactly 0.
        # one-time initialization memsets go to GPSIMD: it is idle, and at
        # startup there is no DVE traffic to contend with
        zsnap = consts.tile([HA, CFREE], BF16, tag="zsnap")
        nc.gpsimd.memset(zsnap[:], 0.0)
        NSNAP = 11
        snap_ring = []
        for i in range(NSNAP):
            s = snapsp.tile([HA, CFREE], BF16, tag=f"snap{i}", name=f"snap{i}")
            # only rows 80.. of the B half must be (and stay) zero; the rest
            # is fully rewritten every slice (partition start must be
            # 32-aligned, so start at 64)
            nc.gpsimd.memset(s[64:HA, HALF:CFREE], 0.0)
            snap_ring.append(s)

        # ping-pong diff tiles; cols 0..8 and 1009..1017 stay zero
        dd_tiles = []
        for i in range(2):
            t = ddp.tile([HA, DDW], BF16, tag=f"dd{i}", name=f"dd{i}")
            nc.gpsimd.memset(t[:, 0:9], 0.0)
            nc.gpsimd.memset(t[:, 9 + HALF : 18 + HALF], 0.0)
            dd_tiles.append(t)

        # chan tiles pre-zeroed so the first squares pass reads no
        # uninitialized SBUF in rows 88..127 of the B half
        chan_ring = []
        for i in range(4):
            t = chans.tile([KT, CFREE], BF16, tag=f"chan{i}", name=f"chan{i}")
            nc.gpsimd.memset(t[64:KT, HALF:CFREE], 0.0)
            chan_ring.append(t)

        def h_pass(z):
            chan = chan_ring[z % 4]
            # all 5 channels [I, J, I^2, J^2, IJ] are host-precomputed:
            # one DMA per h-chunk lands the full channel block (2.5x the
            # bytes of the 2-channel scheme, but the DMA engines are idle
            # and this deletes all per-slice DVE prep work)
            nc.sync.dma_start(chan[0:KT, 0:HALF], vol[z, 0:KT])
            nc.sync.dma_start(chan[0:KB, HALF:CFREE], vol[z, HE - KB : HE])

            start = z == 0
            nc.tensor.matmul(
                psA3[:, 0, 0:500], band[0:120, 0:HA], chan[0:120, 0:500],
                start=start, stop=True, skip_group_check=True,
            )
            nc.tensor.matmul(
                psA3[:, 1, 0:500], band[0:120, 0:HA], chan[0:120, 500:1000],
                start=start, stop=True, skip_group_check=True,
            )
            nc.tensor.matmul(
                psB3[:, 0, 0:500], band[0:KB, 0:HB],
                chan[0:KB, HALF : HALF + 500],
                start=start, stop=True, skip_group_check=True,
            )
            nc.tensor.matmul(
                psB3[:, 1, 0:500], band[0:KB, 0:HB],
                chan[0:KB, HALF + 500 : CFREE],
                start=start, stop=True, skip_group_check=True,
            )

            # both PSUM evacuations on ACT (closer to PSUM; DVE is the
            # bottleneck engine)
            snap = snap_ring[z % NSNAP]
            s4 = snap.rearrange("p (g b w) -> p g b w", g=2, b=2)
            nc.scalar.copy(s4[:, 0], psA3[:, :, 0:500])
            nc.scalar.copy(s4[0:HB, 1], psB3[:, :, 0:500])

        wf_ring = {}

        def w_pass(oz):
            hi = snap_ring[(oz + 8) % NSNAP]
            lo = zsnap if oz == 0 else snap_ring[(oz - 1) % NSNAP]
            dd = dd_tiles[oz % 2]

            # D window = C[oz+8] - C[oz-1], computed on the idle TensorE as
            # dd = I*hi + (-I)*lo accumulated in PSUM banks 4-7, evacuated
            # by ScalarE. Frees ~28us of DVE (the bottleneck engine).
            # hi/lo interleaved per bank so each bank's group closes as
            # early as possible and the evac can start after 4 matmuls.
            for t3, base in ((ddpsA3, 0), (ddpsB3, 2)):
                for p in range(2):
                    sl = slice((base + p) * 500, (base + p + 1) * 500)
                    nc.tensor.matmul(
                        t3[:, p, 0:500], identP, hi[:, sl],
                        start=True, stop=False, skip_group_check=True,
                    )
                    nc.tensor.matmul(
                        t3[:, p, 0:500], identN, lo[:, sl],
                        start=False, stop=True, skip_group_check=True,
                    )

        def ev_scan_pass(oz):
            # evac + scan for oz, one iteration after its diff matmuls:
            # the ScalarE copies issue with their PE deps already complete
            dd = dd_tiles[oz % 2]
            ddv1 = dd[:, 9 : 9 + HALF].rearrange("p (b w) -> p b w", b=2)
            ddv2 = dd[:, 18 + HALF : DDW].rearrange("p (b w) -> p b w", b=2)
            nc.scalar.copy(ddv1, ddpsA3[:, :, 0:500])
            nc.scalar.copy(ddv2, ddpsB3[:, :, 0:500])

            # W window sum: s[t] = s[t-1] + dd[t] - dd[t-9] (f32 state keeps
            # the telescoping exact; a pre-rounded bf16 g would random-walk)
            wf = wfs.tile([HA, DDW], BF16, tag="wf", name="wf")
            nc.vector.tensor_tensor_scan(
                wf[:, 0:SCW], dd[:, 9:DDW], dd[:, 0:SCW], 0.0,
                AOT.add, AOT.subtract,
            )

            wf_ring[oz % 3] = wf

        def cc_pass(oz):
            # software-pipelined one iteration behind the scan, so these
            # ACT ops never head-of-line-block the next slice's snapshot
            # copies while waiting on an in-flight scan
            wf = wf_ring[oz % 3]
            wfh = wf.rearrange("p (h q) -> p h q", h=2)  # [112, 2, 1009]

            def F(c):
                return wfh[:, :, c * WE + 8 : c * WE + 200]

            sc = ccs.tile([HA, 4 * 384], BF16, tag="sc", name="sc")
            sc4 = sc.rearrange("p (s h x) -> p s h x", s=4, h=2)
            t1, s12, sg1, sg2 = (sc4[:, i] for i in range(4))
            sq = ccs.tile([HA, 2 * 384], BF16, tag="sq", name="sq")
            sq2v = sq.rearrange("p (s h x) -> p s h x", s=2, h=2)
            denf = ccs.tile([HA, 384], F32, tag="denf", name="denf")
            rdenf = ccs.tile([HA, 384], F32, tag="rdenf", name="rdenf")
            numf = ccs.tile([HA, 384], F32, tag="numf", name="numf")
            ccv = ccs.tile([HA, 384], BF16, tag="ccv", name="ccv")
            den3 = denf.rearrange("p (h x) -> p h x", h=2)
            rden3 = rdenf.rearrange("p (h x) -> p h x", h=2)
            num3 = numf.rearrange("p (h x) -> p h x", h=2)
            ccv3 = ccv.rearrange("p (h x) -> p h x", h=2)

            nc.vector.tensor_tensor(t1, F(0), F(1), AOT.mult)
            nc.vector.tensor_tensor(s12, F(4), t1, AOT.subtract)
            nc.scalar.activation(sq2v[:, 0], F(0), ACTF.Square)
            nc.scalar.activation(sq2v[:, 1], F(1), ACTF.Square)
            nc.vector.tensor_tensor(sg1, F(2), sq2v[:, 0], AOT.subtract)
            nc.vector.tensor_tensor(sg2, F(3), sq2v[:, 1], AOT.subtract)
            # den = (sg1 + eps) * sg2 > 0 on real rows; the all-zero B-half
            # rows 80..111 give den = 0 -> rden = NaN, but those rows are
            # excluded from the accumulation below.
            nc.vector.scalar_tensor_tensor(
                den3, sg1, EPS, sg2, AOT.add, AOT.mult
            )
            nc.vector.reciprocal_approx_fast(rdenf[:], denf[:])
            nc.scalar.activation(num3, s12, ACTF.Square)
            # cc = num * rden, accumulated per partition; halves separately
            # so the zero/garbage B-half rows 80..111 are never read
            nc.vector.scalar_tensor_tensor(
                ccv3[:, 0], num3[:, 0], 1.0, rden3[:, 0], AOT.mult, AOT.mult,
                accum_out=acc[:, 2 * oz : 2 * oz + 1],
            )
            nc.vector.scalar_tensor_tensor(
                ccv3[0:HB, 1], num3[0:HB, 1], 1.0, rden3[0:HB, 1],
                AOT.mult, AOT.mult,
                accum_out=acc[0:HB, 2 * oz + 1 : 2 * oz + 2],
            )

        for z in range(din):
            h_pass(z)
            oz = z - 8
            if 1 <= oz:
                ev_scan_pass(oz - 1)
            if 0 <= oz < dout:
                w_pass(oz)
            if 2 <= oz:
                cc_pass(oz - 2)
        ev_scan_pass(dout - 1)
        cc_pass(dout - 2)
        cc_pass(dout - 1)

        nc.sync.dma_start(out_d.ap(), acc[:])

    nc.compile()
    return nc


_PROGRAM_CACHE = {}


def _get_program(din, dout):
    key = (din, dout)
    if key not in _PROGRAM_CACHE:
        _PROGRAM_CACHE[key] = build_program(din, dout)
    return _PROGRAM_CACHE[key]


def kernel(pred, target):
    import ml_dtypes

    pred = np.asarray(pred).reshape(D_TOT, H, W).astype(np.float32)
    targ = np.asarray(target).reshape(D_TOT, H, W).astype(np.float32)

    dout = D_TOT // N_CORES
    din = dout + 2 * PAD

    # shifted (x - 0.5) volume; pad value -0.5 == shifted zero-pad (exact).
    # All 5 channels are precomputed here (squaring the bf16-rounded values
    # bit-matches the previous on-chip prep).
    dpad = D_TOT + 2 * PAD
    Ip = np.full((dpad, HE, WE), -0.5, np.float32)
    Jp = np.full((dpad, HE, WE), -0.5, np.float32)
    Ip[PAD:-PAD, PAD : PAD + H, PAD : PAD + W] = targ - 0.5
    Jp[PAD:-PAD, PAD : PAD + H, PAD : PAD + W] = pred - 0.5
    Ib = Ip.astype(ml_dtypes.bfloat16)
    Jb = Jp.astype(ml_dtypes.bfloat16)
    If = Ib.astype(np.float32)
    Jf = Jb.astype(np.float32)
    vol = np.empty((dpad, HE, NCH, WE), ml_dtypes.bfloat16)
    vol[:, :, 0] = Ib
    vol[:, :, 1] = Jb
    vol[:, :, 2] = (If * If).astype(ml_dtypes.bfloat16)
    vol[:, :, 3] = (Jf * Jf).astype(ml_dtypes.bfloat16)
    vol[:, :, 4] = (If * Jf).astype(ml_dtypes.bfloat16)

    band, ident = make_consts()
    nc = _get_program(din, dout)

    in_maps = []
    for c in range(N_CORES):
        s = c * dout
        in_maps.append(
            {
                "vol": np.ascontiguousarray(vol[s : s + din]),
                "band": band,
                "ident": ident,
            }
        )

    res = run_bass_kernel_spmd(nc, in_maps, core_ids=list(range(N_CORES)))
    total = sum(float(r["out"].astype(np.float64).sum()) for r in res.results)
    return np.float32(1.0 - total / float(D_TOT * H * W))
